# revision 3
# baseline (speedup 1.0000x reference)
"""Trainium2 Bass kernel for nn_CrossAttention_50251117363418.

Cross-attention with GQA (24 q heads, 8 kv heads, head_dim 128) + RMS-norm
on q/k, over b=2, nq=2048, nkv=1024, hid=3072, fp32.

Sharding: tensor-parallel over heads across 8 NeuronCores. Core c owns
q heads 3c..3c+2 (Wq rows 384c:384c+384), kv head c (Wk/Wv rows
128c:128c+128), and Wo columns 384c:384c+384. Each core computes a full
[4096, 3072] partial of the output projection; the host sums the 8
partials (the Wo contraction dim is sharded).

Device pipeline per core (all matmuls in float32r, ~1.6e-4 rel err):
  B:  K/V projection from F^T tiles -> K [j,d] (rms-normed) -> PE-transpose
      -> K^T [d,j]; V [j,d] kept natural.
  A(b): Q^T projection (lhsT=Wq^T c-tiles, rhs=X^T c-tiles) -> Q^T [d,i]
      in PSUM; rms-norm over d via ones-matmul (partition-replicated
      column sums of q^2) + reciprocal + sqrt; scale folded in.
  C(b): S^T = K^T tiles (stationary) x Q^T (moving) -> exp on ACT ->
      E^T [j,i]; column sums via ones-matmul (partition-replicated);
      O^T = V tiles x E^T accumulated over j; normalize O^T by the
      replicated reciprocal sums (softmax max-subtraction is skipped:
      rms-normed logits are O(5), well within exp range).
  D(b): out partial = O^T tiles (stationary) x Wo^T (moving), 3 heads
      accumulated in PSUM, copied + DMA'd out.

Inputs X^T, F^T and all weights are pre-transposed/sliced on the host
(layout prep for the chosen sharding), so no X/F transposes on device.
"""
import sys

sys.path.insert(0, "/opt/trn_rl_repo")

import numpy as np

B, NQ, NKV, HID = 2, 2048, 1024, 3072
NH, NKVH, DK = 24, 8, 128
NCORES = 8
HPC = NH // NCORES  # q heads per core = 3
QO = HPC * DK  # per-core q out dim = 384
NR = B * NQ  # 4096 q rows
FR = B * NKV  # 2048 kv rows
CT = HID // 128  # 24 contraction tiles
EPS = 1e-6
SCALE2 = 1.0 / DK  # (HEAD_DIM ** -0.5) ** 2, folded into sqrt scale

_COMPILED = None


def _build():
    import concourse.bass as bass
    import concourse.mybir as mybir
    from concourse import bacc
    from concourse.tile import TileContext
    from concourse.masks import make_identity

    f32 = mybir.dt.float32
    f32r = mybir.dt.float32r
    MULT = mybir.AluOpType.mult
    ADD = mybir.AluOpType.add
    EXP = mybir.ActivationFunctionType.Exp
    SQRT = mybir.ActivationFunctionType.Sqrt
    SQUARE = mybir.ActivationFunctionType.Square

    nc = bacc.Bacc(None, target_bir_lowering=False)

    xt = nc.declare_dram_parameter("xt", [HID, NR], f32r, isOutput=False)
    ft = nc.declare_dram_parameter("ft", [HID, FR], f32r, isOutput=False)
    wqt = nc.declare_dram_parameter("wqt", [HID, QO], f32r, isOutput=False)
    wkvt = nc.declare_dram_parameter("wkvt", [HID, 2 * DK], f32r, isOutput=False)
    wot = nc.declare_dram_parameter("wot", [QO, HID], f32r, isOutput=False)
    out = nc.declare_dram_parameter("out", [NR, HID], f32, isOutput=True)

    xt_r = xt[:, :].rearrange("(a p) n -> p a n", p=128)
    ft_r = ft[:, :].rearrange("(a p) n -> p a n", p=128)
    wqt_r = wqt[:, :].rearrange("(a p) n -> p a n", p=128)
    wkvt_r = wkvt[:, :].rearrange("(a p) n -> p a n", p=128)
    wot_r = wot[:, :].rearrange("(a p) n -> p a n", p=128)

    IC_A = 256  # phase-A i-chunk (moving dim)
    NCH_A = NQ // IC_A  # 8 chunks per batch
    IC_C = 512  # phase-C i-chunk
    NCH_C = NQ // IC_C  # 4 chunks per batch
    JT = NKV // 128  # 8 j-tiles per batch
    HC = 512  # phase-D hid chunk
    NHC = HID // HC  # 6

    with TileContext(nc) as tc:
        with (
            tc.tile_pool(name="glob", bufs=1) as glob,
            tc.tile_pool(name="gps", bufs=2, space="PSUM") as gps,
        ):
            ones_f = glob.tile([128, 128], f32)
            nc.gpsimd.memset(ones_f, 1.0)
            ones = glob.tile([128, 128], f32r)
            nc.vector.tensor_copy(ones, ones_f)
            ident_f = glob.tile([128, 128], f32)
            make_identity(nc, ident_f)
            ident = glob.tile([128, 128], f32r)
            nc.vector.tensor_copy(ident, ident_f)

            wqt_sb = glob.tile([128, CT, QO], f32r)
            nc.sync.dma_start(out=wqt_sb, in_=wqt_r)
            wkvt_sb = glob.tile([128, CT, 2 * DK], f32r)
            nc.sync.dma_start(out=wkvt_sb, in_=wkvt_r)
            wot_sb = glob.tile([128, HPC, HID], f32r)
            nc.sync.dma_start(out=wot_sb, in_=wot_r)

            kt_sb = glob.tile([128, B * JT, 128], f32r)  # K^T [d, j]
            v_sb = glob.tile([128, B * JT, 128], f32r)  # V [j, d]

            # ---- Phase B: K/V projection + k-norm + K transpose ----
            with (
                tc.tile_pool(name="bwork", bufs=2) as bwork,
                tc.tile_pool(name="bw1", bufs=3) as bw1,
                tc.tile_pool(name="bps", bufs=2, space="PSUM") as bps,
            ):
                for jt in range(B * JT):
                    ftc = bwork.tile([128, CT, 128], f32r)
                    nc.sync.dma_start(
                        out=ftc, in_=ft_r[:, :, jt * 128 : (jt + 1) * 128]
                    )
                    pkv = bps.tile([128, 2 * DK], f32)
                    for ct in range(CT):
                        nc.tensor.matmul(
                            pkv,
                            ftc[:, ct, :],
                            wkvt_sb[:, ct, :],
                            start=(ct == 0),
                            stop=(ct == CT - 1),
                        )
                    kraw = pkv[:, 0:DK]
                    sqk = bw1.tile([128, DK], f32)
                    ssq = bw1.tile([128, 1], f32)
                    nc.scalar.activation(sqk, kraw, SQUARE, accum_out=ssq)
                    uk = bw1.tile([128, 1], f32)
                    nc.vector.tensor_scalar(uk, ssq, 1.0 / DK, EPS, MULT, ADD)
                    rk = bw1.tile([128, 1], f32)
                    nc.vector.reciprocal(rk, uk)
                    gk = bw1.tile([128, 1], f32)
                    nc.scalar.activation(gk, rk, SQRT)
                    khat = bw1.tile([128, DK], f32r)
                    nc.vector.tensor_scalar(khat, kraw, gk, None, MULT)
                    nc.vector.tensor_copy(v_sb[:, jt, :], pkv[:, DK : 2 * DK])
                    pkt = bps.tile([128, DK], f32r)
                    nc.tensor.transpose(pkt, khat, ident)
                    nc.vector.tensor_copy(kt_sb[:, jt, :], pkt)

            # ---- Per-batch: A (q proj+norm), C (attention), D (out proj) ----
            for b in range(B):
                with tc.tile_pool(name="qtp", bufs=1) as qtp:
                    qt_sb = qtp.tile([128, HPC, NQ], f32r)  # Q^T [d, h, i]

                    with (
                        tc.tile_pool(name="awork", bufs=2) as awork,
                        tc.tile_pool(name="aw1", bufs=3) as aw1,
                        tc.tile_pool(name="aps", bufs=2, space="PSUM") as aps,
                        tc.tile_pool(name="aps2", bufs=2, space="PSUM") as aps2,
                    ):
                        for ic in range(NCH_A):
                            i0 = b * NQ + ic * IC_A
                            xtc = awork.tile([128, CT, IC_A], f32r)
                            nc.sync.dma_start(
                                out=xtc, in_=xt_r[:, :, i0 : i0 + IC_A]
                            )
                            for o in range(HPC):
                                pq = aps.tile([128, IC_A], f32)
                                for ct in range(CT):
                                    nc.tensor.matmul(
                                        pq,
                                        wqt_sb[:, ct, o * 128 : (o + 1) * 128],
                                        xtc[:, ct, :],
                                        start=(ct == 0),
                                        stop=(ct == CT - 1),
                                    )
                                qraw = aw1.tile([128, IC_A], f32)
                                nc.vector.tensor_copy(qraw, pq)
                                sq = aw1.tile([128, IC_A], f32r)
                                nc.vector.tensor_tensor(sq, qraw, qraw, MULT)
                                psums = aps2.tile([128, IC_A], f32)
                                nc.tensor.matmul(
                                    psums, ones, sq, start=True, stop=True
                                )
                                u = aw1.tile([128, IC_A], f32)
                                nc.vector.tensor_scalar(
                                    u, psums, 1.0 / DK, EPS, MULT, ADD
                                )
                                rv = aw1.tile([128, IC_A], f32)
                                nc.vector.reciprocal(rv, u)
                                g = aw1.tile([128, IC_A], f32)
                                nc.scalar.activation(g, rv, SQRT, scale=SCALE2)
                                nc.vector.tensor_tensor(
                                    qt_sb[:, o, ic * IC_A : (ic + 1) * IC_A],
                                    qraw,
                                    g,
                                    MULT,
                                )

                    # ---- Phase C: attention for this batch ----
                    with tc.tile_pool(name="otp", bufs=1) as otp:
                        ot_sb = otp.tile([128, HPC, NQ], f32r)  # O^T [d, h, i]

                        with (
                            tc.tile_pool(name="cwork", bufs=2) as cwork,
                            tc.tile_pool(name="cw1", bufs=2) as cw1,
                            tc.tile_pool(name="cst", bufs=3, space="PSUM") as cst,
                            tc.tile_pool(name="csum", bufs=2, space="PSUM") as csum,
                            tc.tile_pool(name="co", bufs=2, space="PSUM") as co,
                        ):
                            for h in range(HPC):
                                for ic in range(NCH_C):
                                    isl = slice(ic * IC_C, (ic + 1) * IC_C)
                                    e_sb = cwork.tile([128, JT, IC_C], f32r)
                                    for jt in range(JT):
                                        pst = cst.tile([128, IC_C], f32)
                                        nc.tensor.matmul(
                                            pst,
                                            kt_sb[:, b * JT + jt, :],
                                            qt_sb[:, h, isl],
                                            start=True,
                                            stop=True,
                                        )
                                        nc.scalar.activation(
                                            e_sb[:, jt, :], pst, EXP
                                        )
                                    psum = csum.tile([128, IC_C], f32)
                                    po = co.tile([128, IC_C], f32)
                                    for jt in range(JT):
                                        nc.tensor.matmul(
                                            psum,
                                            ones,
                                            e_sb[:, jt, :],
                                            start=(jt == 0),
                                            stop=(jt == JT - 1),
                                        )
                                    for jt in range(JT):
                                        nc.tensor.matmul(
                                            po,
                                            v_sb[:, b * JT + jt, :],
                                            e_sb[:, jt, :],
                                            start=(jt == 0),
                                            stop=(jt == JT - 1),
                                        )
                                    rs = cw1.tile([128, IC_C], f32)
                                    nc.vector.reciprocal(rs, psum)
                                    nc.vector.tensor_tensor(
                                        ot_sb[:, h, isl], po, rs, MULT
                                    )

                        # ---- Phase D: output projection for this batch ----
                        with (
                            tc.tile_pool(name="dwork", bufs=2) as dwork,
                            tc.tile_pool(name="dps", bufs=4, space="PSUM") as dps,
                        ):
                            for it in range(NQ // 128):
                                obuf = dwork.tile([128, HID], f32)
                                for hc in range(NHC):
                                    pu = dps.tile([128, HC], f32)
                                    for h in range(HPC):
                                        nc.tensor.matmul(
                                            pu,
                                            ot_sb[:, h, it * 128 : (it + 1) * 128],
                                            wot_sb[:, h, hc * HC : (hc + 1) * HC],
                                            start=(h == 0),
                                            stop=(h == HPC - 1),
                                        )
                                    nc.vector.tensor_copy(
                                        obuf[:, hc * HC : (hc + 1) * HC], pu
                                    )
                                r0 = b * NQ + it * 128
                                nc.sync.dma_start(
                                    out=out[r0 : r0 + 128, :], in_=obuf
                                )

    nc.compile()
    return nc


def _numpy_reference(hidden_states, v_features, attention_mask, Wq, Wk, Wv, Wo,
                     q_norm_w, k_norm_w):
    """Pure-numpy fallback replicating reference() for unexpected inputs."""
    x = hidden_states.astype(np.float64)
    f = v_features.astype(np.float64)
    b, nq, hid = x.shape
    nkv = f.shape[1]
    groups = NH // NKVH
    scale = DK ** -0.5

    def rms(t, w):
        var = (t * t).mean(-1, keepdims=True)
        return w * (t / np.sqrt(var + EPS))

    q = np.einsum("bnc,oc->bno", x, Wq.astype(np.float64))
    q = q.reshape(b, nq, NH, DK).transpose(0, 2, 1, 3)
    q = rms(q, q_norm_w) * scale
    k = np.einsum("bnc,oc->bno", f, Wk.astype(np.float64))
    k = k.reshape(b, nkv, NKVH, DK).transpose(0, 2, 1, 3)
    v = np.einsum("bnc,oc->bno", f, Wv.astype(np.float64))
    v = v.reshape(b, nkv, NKVH, DK).transpose(0, 2, 1, 3)
    k = rms(k, k_norm_w)
    k = np.repeat(k, groups, axis=1)
    v = np.repeat(v, groups, axis=1)
    sim = np.einsum("bhid,bhjd->bhij", q, k)
    mask = attention_mask.astype(bool)[:, None, :, None]
    sim = np.where(mask, sim, np.finfo(np.float32).min)
    sim = sim - sim.max(-1, keepdims=True)
    e = np.exp(sim)
    attn = e / e.sum(-1, keepdims=True)
    o = np.einsum("bhij,bhjd->bhid", attn, v)
    o = o.transpose(0, 2, 1, 3).reshape(b, nq, NH * DK)
    return np.einsum("bno,co->bnc", o, Wo.astype(np.float64)).astype(np.float32)


def kernel(**inputs):
    hidden_states = np.asarray(inputs["hidden_states"], dtype=np.float32)
    v_features = np.asarray(inputs["v_features"], dtype=np.float32)
    attention_mask = np.asarray(inputs["attention_mask"])
    Wq = np.asarray(inputs["Wq"], dtype=np.float32)
    Wk = np.asarray(inputs["Wk"], dtype=np.float32)
    Wv = np.asarray(inputs["Wv"], dtype=np.float32)
    Wo = np.asarray(inputs["Wo"], dtype=np.float32)
    q_norm_w = np.asarray(inputs["q_norm_w"], dtype=np.float32)
    k_norm_w = np.asarray(inputs["k_norm_w"], dtype=np.float32)

    if (not attention_mask.all()) or (q_norm_w != 1.0).any() or (
        k_norm_w != 1.0
    ).any():
        # Device kernel hardcodes all-ones mask and unit norm weights
        # (guaranteed by the problem spec); fall back for anything else.
        return _numpy_reference(
            hidden_states, v_features, attention_mask, Wq, Wk, Wv, Wo,
            q_norm_w, k_norm_w,
        )

    global _COMPILED
    if _COMPILED is None:
        _COMPILED = _build()
    nc = _COMPILED

    xt = np.ascontiguousarray(hidden_states.reshape(NR, HID).T)
    ft = np.ascontiguousarray(v_features.reshape(FR, HID).T)

    in_maps = []
    for c in range(NCORES):
        qsl = slice(c * QO, (c + 1) * QO)
        ksl = slice(c * DK, (c + 1) * DK)
        in_maps.append(
            {
                "xt": xt,
                "ft": ft,
                "wqt": np.ascontiguousarray(Wq[qsl, :].T),
                "wkvt": np.ascontiguousarray(
                    np.concatenate([Wk[ksl, :], Wv[ksl, :]], axis=0).T
                ),
                "wot": np.ascontiguousarray(Wo[:, qsl].T),
            }
        )

    from concourse.bass_utils import run_bass_kernel_spmd

    res = run_bass_kernel_spmd(nc, in_maps, list(range(NCORES)))
    acc = res.results[0]["out"].astype(np.float32)
    for c in range(1, NCORES):
        acc = acc + res.results[c]["out"]
    return acc.reshape(B, NQ, HID)


# revision 8
# speedup vs baseline: 1.3031x; 1.3031x over previous
"""Trainium2 Bass kernel for nn_CrossAttention_50251117363418.

Cross-attention with GQA (24 q heads, 8 kv heads, head_dim 128) + RMS-norm
on q/k, over b=2, nq=2048, nkv=1024, hid=3072, fp32.

Sharding: tensor-parallel over heads across 8 NeuronCores. Core c owns
q heads 3c..3c+2 (Wq rows 384c:384c+384), kv head c (Wk/Wv rows
128c:128c+128), and Wo columns 384c:384c+384. Each core computes a full
[4096, 3072] partial of the output projection; the host sums the 8
partials (the Wo contraction dim is sharded).

Device pipeline per core (all matmuls in float32r, ~1.6e-4 rel err):
  B:  K/V projection from F^T tiles -> K [j,d] (rms-normed) -> PE-transpose
      -> K^T [d,j]; V [j,d] kept natural. Interleaved chunk-wise with A(0)
      so the startup is not DMA-serialized.
  A(b): Q^T projection (lhsT=Wq^T c-tiles, rhs=X^T c-tiles) -> Q^T [d,i]
      in PSUM; rms-norm over d via ones-matmul (partition-replicated
      column sums of q^2, squares computed on ACT straight from PSUM);
      software-pipelined so the PE never waits on the norm chain.
  C(b): S^T = K^T tiles (stationary) x Q^T (moving) -> exp on ACT ->
      E^T [j,i]; column sums via ones-matmul (partition-replicated);
      O^T = V tiles x E^T accumulated over j; normalize O^T by the
      replicated reciprocal sums (softmax max-subtraction is skipped:
      rms-normed logits are O(5), well within exp range).
  D(b): out partial = O^T tiles (stationary) x Wo^T (moving), 3 heads
      accumulated in PSUM, copied on ACT + DMA'd out on the scalar
      HWDGE queue (so output DMA never blocks input prefetch).

Inputs X^T, F^T and all weights are pre-transposed/sliced on the host
(layout prep for the chosen sharding), so no X/F transposes on device.
"""
import sys
from collections import deque

sys.path.insert(0, "/opt/trn_rl_repo")

import numpy as np

B, NQ, NKV, HID = 2, 2048, 1024, 3072
NH, NKVH, DK = 24, 8, 128
NCORES = 8
HPC = NH // NCORES  # q heads per core = 3
QO = HPC * DK  # per-core q out dim = 384
NR = B * NQ  # 4096 q rows
FR = B * NKV  # 2048 kv rows
CT = HID // 128  # 24 contraction tiles
EPS = 1e-6
SCALE2 = 1.0 / DK  # (HEAD_DIM ** -0.5) ** 2, folded into sqrt scale

_COMPILED = None


def _build():
    import concourse.bass as bass
    import concourse.mybir as mybir
    from concourse import bacc
    from concourse.tile import TileContext
    from concourse.masks import make_identity

    f32 = mybir.dt.float32
    f32r = mybir.dt.float32r
    MULT = mybir.AluOpType.mult
    ADD = mybir.AluOpType.add
    EXP = mybir.ActivationFunctionType.Exp
    SQRT = mybir.ActivationFunctionType.Sqrt
    SQUARE = mybir.ActivationFunctionType.Square
    COPYF = mybir.ActivationFunctionType.Copy

    nc = bacc.Bacc(None, target_bir_lowering=False)

    xt = nc.declare_dram_parameter("xt", [HID, NR], f32r, isOutput=False)
    ft = nc.declare_dram_parameter("ft", [HID, FR], f32r, isOutput=False)
    wqt = nc.declare_dram_parameter("wqt", [HID, QO], f32r, isOutput=False)
    wkvt = nc.declare_dram_parameter("wkvt", [HID, 2 * DK], f32r, isOutput=False)
    wot = nc.declare_dram_parameter("wot", [QO, HID], f32r, isOutput=False)
    out = nc.declare_dram_parameter("out", [NR, HID], f32, isOutput=True)

    xt_r = xt[:, :].rearrange("(a p) n -> p a n", p=128)
    ft_r = ft[:, :].rearrange("(a p) n -> p a n", p=128)
    wqt_r = wqt[:, :].rearrange("(a p) n -> p a n", p=128)
    wkvt_r = wkvt[:, :].rearrange("(a p) n -> p a n", p=128)
    wot_r = wot[:, :].rearrange("(a p) n -> p a n", p=128)

    IC_A = 256  # phase-A i-chunk (moving dim)
    NCH_A = NQ // IC_A  # 8 chunks per batch
    IC_C = 512  # phase-C i-chunk
    NCH_C = NQ // IC_C  # 4 chunks per batch
    JT = NKV // 128  # 8 j-tiles per batch
    HC = 512  # phase-D hid chunk
    NHC = HID // HC  # 6

    with TileContext(nc) as tc:
        with tc.tile_pool(name="glob", bufs=1) as glob:
            ones_f = glob.tile([128, 128], f32)
            nc.gpsimd.memset(ones_f, 1.0)
            ones = glob.tile([128, 128], f32r)
            nc.vector.tensor_copy(ones, ones_f)
            ident_f = glob.tile([128, 128], f32)
            make_identity(nc, ident_f)
            ident = glob.tile([128, 128], f32r)
            nc.vector.tensor_copy(ident, ident_f)

            wqt_sb = glob.tile([128, CT, QO], f32r)
            kt_sb = glob.tile([128, B * JT, 128], f32r)  # K^T [d, j]
            v_sb = glob.tile([128, B * JT, 128], f32r)  # V [j, d]

            for b in range(B):
                with tc.tile_pool(name="qtp", bufs=1) as qtp:
                    qt_sb = qtp.tile([128, HPC, NQ], f32r)  # Q^T [d, h, i]

                    with (
                        tc.tile_pool(name="awork", bufs=2) as awork,
                        tc.tile_pool(name="aw1", bufs=2) as aw1,
                        tc.tile_pool(name="aqr", bufs=3) as aqr,
                        tc.tile_pool(name="aps", bufs=2, space="PSUM") as aps,
                        tc.tile_pool(name="aps2", bufs=2, space="PSUM") as aps2,
                        _b_pools(tc, b) as bp,
                    ):
                        # pending A-norm pipeline state
                        pend = deque()

                        def emit_a_mms(ic):
                            i0 = b * NQ + ic * IC_A
                            xtc = awork.tile([128, CT, IC_A], f32r, name="xtc")
                            nc.sync.dma_start(
                                out=xtc, in_=xt_r[:, :, i0 : i0 + IC_A]
                            )
                            for o in range(HPC):
                                pq = aps.tile([128, IC_A], f32, name="pq")
                                for ct in range(CT):
                                    nc.tensor.matmul(
                                        pq,
                                        wqt_sb[:, ct, o * 128 : (o + 1) * 128],
                                        xtc[:, ct, :],
                                        start=(ct == 0),
                                        stop=(ct == CT - 1),
                                    )
                                pend.append({"pq": pq, "ic": ic, "o": o, "st": 0})
                                _a_step()

                        def _a_tail1(e):
                            # square + raw copy on ACT (reads PSUM, frees pq),
                            # ones-matmul for replicated column sums, u = var+eps
                            sq = aw1.tile([128, IC_A], f32r, name="sq")
                            nc.scalar.activation(sq, e["pq"], SQUARE)
                            qraw = aqr.tile([128, IC_A], f32, name="qraw")
                            nc.scalar.activation(qraw, e["pq"], COPYF)
                            psums = aps2.tile([128, IC_A], f32, name="psums")
                            nc.tensor.matmul(psums, ones, sq, start=True, stop=True)
                            u = aw1.tile([128, IC_A], f32, name="u")
                            nc.vector.tensor_scalar(
                                u, psums, 1.0 / DK, EPS, MULT, ADD
                            )
                            rv = aw1.tile([128, IC_A], f32, name="rv")
                            nc.vector.reciprocal(rv, u)
                            e["qraw"], e["rv"] = qraw, rv

                        def _a_tail2(e):
                            g = aw1.tile([128, IC_A], f32, name="g")
                            nc.scalar.activation(g, e["rv"], SQRT, scale=SCALE2)
                            ic, o = e["ic"], e["o"]
                            nc.vector.tensor_tensor(
                                qt_sb[:, o, ic * IC_A : (ic + 1) * IC_A],
                                e["qraw"],
                                g,
                                MULT,
                            )

                        def _a_step():
                            # tail1 lags the matmuls by 1 group, tail2 by 2,
                            # so the PE never waits on ACT/DVE and the ACT
                            # stream never waits on the DVE reciprocal.
                            if len(pend) >= 2 and pend[-2]["st"] == 0:
                                _a_tail1(pend[-2])
                                pend[-2]["st"] = 1
                            if len(pend) >= 3 and pend[-3]["st"] == 1:
                                _a_tail2(pend[-3])
                                pend[-3]["st"] = 2
                            while pend and pend[0]["st"] == 2:
                                pend.popleft()

                        if b == 0:
                            for k in range(NCH_A):
                                _emit_b_mms(nc, bp, ft_r, wkvt_r, 2 * k)
                                _emit_b_mms(nc, bp, ft_r, wkvt_r, 2 * k + 1)
                                if k == 0:
                                    nc.sync.dma_start(out=wqt_sb, in_=wqt_r)
                                emit_a_mms(k)
                                _emit_b_tail(
                                    nc, bp, kt_sb, v_sb, ident, 2 * k, mybir
                                )
                                _emit_b_tail(
                                    nc, bp, kt_sb, v_sb, ident, 2 * k + 1, mybir
                                )
                        else:
                            for k in range(NCH_A):
                                emit_a_mms(k)
                        for e in list(pend):
                            if e["st"] == 0:
                                _a_tail1(e)
                                e["st"] = 1
                        for e in list(pend):
                            if e["st"] == 1:
                                _a_tail2(e)
                                e["st"] = 2
                        pend.clear()

                    # ---- Phase C: attention + Phase D: out proj ----
                    with tc.tile_pool(name="otp", bufs=1) as otp:
                        ot_sb = otp.tile([128, HPC, NQ], f32r)  # O^T [d, h, i]

                        with tc.tile_pool(name="wotp", bufs=1) as wotp:
                            wot_sb = wotp.tile([128, HPC, HID], f32r)
                            nc.scalar.dma_start(out=wot_sb, in_=wot_r)

                            with (
                                tc.tile_pool(name="cwork", bufs=2) as cwork,
                                tc.tile_pool(name="cw1", bufs=2) as cw1,
                                tc.tile_pool(name="cst", bufs=3, space="PSUM") as cst,
                                tc.tile_pool(name="csum", bufs=2, space="PSUM") as csum,
                                tc.tile_pool(name="co", bufs=2, space="PSUM") as co,
                            ):
                                for h in range(HPC):
                                    for ic in range(NCH_C):
                                        isl = slice(ic * IC_C, (ic + 1) * IC_C)
                                        e_sb = cwork.tile(
                                            [128, JT, IC_C], f32r, name="e_sb"
                                        )
                                        for jt in range(JT):
                                            pst = cst.tile(
                                                [128, IC_C], f32, name="pst"
                                            )
                                            nc.tensor.matmul(
                                                pst,
                                                kt_sb[:, b * JT + jt, :],
                                                qt_sb[:, h, isl],
                                                start=True,
                                                stop=True,
                                            )
                                            nc.scalar.activation(
                                                e_sb[:, jt, :], pst, EXP
                                            )
                                        psum = csum.tile(
                                            [128, IC_C], f32, name="psum"
                                        )
                                        po = co.tile([128, IC_C], f32, name="po")
                                        for jt in range(JT):
                                            nc.tensor.matmul(
                                                psum,
                                                ones,
                                                e_sb[:, jt, :],
                                                start=(jt == 0),
                                                stop=(jt == JT - 1),
                                            )
                                        for jt in range(JT):
                                            nc.tensor.matmul(
                                                po,
                                                v_sb[:, b * JT + jt, :],
                                                e_sb[:, jt, :],
                                                start=(jt == 0),
                                                stop=(jt == JT - 1),
                                            )
                                        rs = cw1.tile([128, IC_C], f32, name="rs")
                                        nc.vector.reciprocal(rs, psum)
                                        nc.vector.tensor_tensor(
                                            ot_sb[:, h, isl], po, rs, MULT
                                        )

                            # ---- Phase D ----
                            with (
                                tc.tile_pool(name="dwork", bufs=2) as dwork,
                                tc.tile_pool(name="dps", bufs=4, space="PSUM") as dps,
                            ):
                                for it in range(NQ // 128):
                                    obuf = dwork.tile([128, HID], f32, name="obuf")
                                    for hc in range(NHC):
                                        pu = dps.tile([128, HC], f32, name="pu")
                                        for h in range(HPC):
                                            nc.tensor.matmul(
                                                pu,
                                                ot_sb[
                                                    :, h, it * 128 : (it + 1) * 128
                                                ],
                                                wot_sb[
                                                    :, h, hc * HC : (hc + 1) * HC
                                                ],
                                                start=(h == 0),
                                                stop=(h == HPC - 1),
                                            )
                                        nc.scalar.activation(
                                            obuf[:, hc * HC : (hc + 1) * HC],
                                            pu,
                                            COPYF,
                                        )
                                    r0 = b * NQ + it * 128
                                    nc.scalar.dma_start(
                                        out=out[r0 : r0 + 128, :], in_=obuf
                                    )

    nc.compile()
    return nc


class _BP:
    pass


def _b_pools(tc, b):
    """Context manager bundling phase-B pools; no-op for b != 0."""
    import contextlib

    if b != 0:
        return contextlib.nullcontext(None)

    @contextlib.contextmanager
    def _cm():
        with (
            tc.tile_pool(name="bwt", bufs=1) as bwt,
            tc.tile_pool(name="bwork", bufs=2) as bwork,
            tc.tile_pool(name="bw1", bufs=2) as bw1,
            tc.tile_pool(name="bps", bufs=2, space="PSUM") as bps,
        ):
            bp = _BP()
            bp.bwt, bp.bwork, bp.bw1, bp.bps = bwt, bwork, bw1, bps
            bp.wkvt_sb = None
            bp.pend = {}
            yield bp

    return _cm()


def _emit_b_mms(nc, bp, ft_r, wkvt_r, jt):
    import concourse.mybir as mybir

    f32 = mybir.dt.float32
    f32r = mybir.dt.float32r
    if bp.wkvt_sb is None:
        # first call: DMA wkvt (before any ft chunk on the sync queue)
        bp.wkvt_sb = bp.bwt.tile([128, CT, 2 * DK], f32r, name="wkvt_sb")
        nc.sync.dma_start(out=bp.wkvt_sb, in_=wkvt_r)
    ftc = bp.bwork.tile([128, CT, 128], f32r, name="ftc")
    nc.sync.dma_start(out=ftc, in_=ft_r[:, :, jt * 128 : (jt + 1) * 128])
    pkv = bp.bps.tile([128, 2 * DK], f32, name="pkv")
    for ct in range(CT):
        nc.tensor.matmul(
            pkv,
            ftc[:, ct, :],
            bp.wkvt_sb[:, ct, :],
            start=(ct == 0),
            stop=(ct == CT - 1),
        )
    bp.pend[jt] = pkv


def _emit_b_tail(nc, bp, kt_sb, v_sb, ident, jt, mybir):
    f32 = mybir.dt.float32
    f32r = mybir.dt.float32r
    MULT = mybir.AluOpType.mult
    ADD = mybir.AluOpType.add
    SQRT = mybir.ActivationFunctionType.Sqrt
    SQUARE = mybir.ActivationFunctionType.Square
    COPYF = mybir.ActivationFunctionType.Copy

    pkv = bp.pend.pop(jt)
    kraw = pkv[:, 0:DK]
    sqk = bp.bw1.tile([128, DK], f32, name="sqk")
    ssq = bp.bw1.tile([128, 1], f32, name="ssq")
    nc.scalar.activation(sqk, kraw, SQUARE, accum_out=ssq)
    uk = bp.bw1.tile([128, 1], f32, name="uk")
    nc.vector.tensor_scalar(uk, ssq, 1.0 / DK, EPS, MULT, ADD)
    rk = bp.bw1.tile([128, 1], f32, name="rk")
    nc.vector.reciprocal(rk, uk)
    gk = bp.bw1.tile([128, 1], f32, name="gk")
    nc.scalar.activation(gk, rk, SQRT)
    khat = bp.bw1.tile([128, DK], f32r, name="khat")
    nc.vector.tensor_scalar(khat, kraw, gk, None, MULT)
    nc.scalar.activation(v_sb[:, jt, :], pkv[:, DK : 2 * DK], COPYF)
    pkt = bp.bps.tile([128, DK], f32r, name="pkt")
    nc.tensor.transpose(pkt, khat, ident)
    nc.vector.tensor_copy(kt_sb[:, jt, :], pkt)


def _numpy_reference(hidden_states, v_features, attention_mask, Wq, Wk, Wv, Wo,
                     q_norm_w, k_norm_w):
    """Pure-numpy fallback replicating reference() for unexpected inputs."""
    x = hidden_states.astype(np.float64)
    f = v_features.astype(np.float64)
    b, nq, hid = x.shape
    nkv = f.shape[1]
    groups = NH // NKVH
    scale = DK ** -0.5

    def rms(t, w):
        var = (t * t).mean(-1, keepdims=True)
        return w * (t / np.sqrt(var + EPS))

    q = np.einsum("bnc,oc->bno", x, Wq.astype(np.float64))
    q = q.reshape(b, nq, NH, DK).transpose(0, 2, 1, 3)
    q = rms(q, q_norm_w) * scale
    k = np.einsum("bnc,oc->bno", f, Wk.astype(np.float64))
    k = k.reshape(b, nkv, NKVH, DK).transpose(0, 2, 1, 3)
    v = np.einsum("bnc,oc->bno", f, Wv.astype(np.float64))
    v = v.reshape(b, nkv, NKVH, DK).transpose(0, 2, 1, 3)
    k = rms(k, k_norm_w)
    k = np.repeat(k, groups, axis=1)
    v = np.repeat(v, groups, axis=1)
    sim = np.einsum("bhid,bhjd->bhij", q, k)
    mask = attention_mask.astype(bool)[:, None, :, None]
    sim = np.where(mask, sim, np.finfo(np.float32).min)
    sim = sim - sim.max(-1, keepdims=True)
    e = np.exp(sim)
    attn = e / e.sum(-1, keepdims=True)
    o = np.einsum("bhij,bhjd->bhid", attn, v)
    o = o.transpose(0, 2, 1, 3).reshape(b, nq, NH * DK)
    return np.einsum("bno,co->bnc", o, Wo.astype(np.float64)).astype(np.float32)


def kernel(**inputs):
    hidden_states = np.asarray(inputs["hidden_states"], dtype=np.float32)
    v_features = np.asarray(inputs["v_features"], dtype=np.float32)
    attention_mask = np.asarray(inputs["attention_mask"])
    Wq = np.asarray(inputs["Wq"], dtype=np.float32)
    Wk = np.asarray(inputs["Wk"], dtype=np.float32)
    Wv = np.asarray(inputs["Wv"], dtype=np.float32)
    Wo = np.asarray(inputs["Wo"], dtype=np.float32)
    q_norm_w = np.asarray(inputs["q_norm_w"], dtype=np.float32)
    k_norm_w = np.asarray(inputs["k_norm_w"], dtype=np.float32)

    if (not attention_mask.all()) or (q_norm_w != 1.0).any() or (
        k_norm_w != 1.0
    ).any():
        # Device kernel hardcodes all-ones mask and unit norm weights
        # (guaranteed by the problem spec); fall back for anything else.
        return _numpy_reference(
            hidden_states, v_features, attention_mask, Wq, Wk, Wv, Wo,
            q_norm_w, k_norm_w,
        )

    global _COMPILED
    if _COMPILED is None:
        _COMPILED = _build()
    nc = _COMPILED

    xt = np.ascontiguousarray(hidden_states.reshape(NR, HID).T)
    ft = np.ascontiguousarray(v_features.reshape(FR, HID).T)

    in_maps = []
    for c in range(NCORES):
        qsl = slice(c * QO, (c + 1) * QO)
        ksl = slice(c * DK, (c + 1) * DK)
        in_maps.append(
            {
                "xt": xt,
                "ft": ft,
                "wqt": np.ascontiguousarray(Wq[qsl, :].T),
                "wkvt": np.ascontiguousarray(
                    np.concatenate([Wk[ksl, :], Wv[ksl, :]], axis=0).T
                ),
                "wot": np.ascontiguousarray(Wo[:, qsl].T),
            }
        )

    from concourse.bass_utils import run_bass_kernel_spmd

    res = run_bass_kernel_spmd(nc, in_maps, list(range(NCORES)))
    acc = res.results[0]["out"].astype(np.float32)
    for c in range(1, NCORES):
        acc = acc + res.results[c]["out"]
    return acc.reshape(B, NQ, HID)
